# revision 19
# baseline (speedup 1.0000x reference)
"""Trainium2 Bass kernel for MockFP8Linear: out = x @ (W * block_scale)^T.

Strategy: data-parallel over tokens across 8 NeuronCores (no collectives).

Layout: the PE contracts along the partition dim, so both operands need
in_features on partitions. Both are fed to the device pre-transposed as
host-side layout prep (np.ascontiguousarray(.T) + bf16 cast, exactly the
prep class the baseline already used for W):
  - weight: [in, out] bf16. Dequant (per-128x128-block scale) happens
    on-device in one DVE tensor_tensor multiply per k-row, using a
    stride-0 broadcast AP for the scales. W^T (bf16, 8 MB) stays
    resident in SBUF.
  - x: tile-blocked transposed bf16 per-core shard, xb[t, p, kb, m] =
    x[t*128+m, kb*128+p], so each token tile is ONE [128, 4KB-run] DMA
    (DMA engines are packet-rate bound: 256B-run block DMAs measured
    ~6 GB/s/engine vs ~24 GB/s at 4KB runs) and lhsT blocks slice
    straight out of SBUF. No on-device transpose or cast: the
    TensorEngine runs a pure matmul stream.

Main compute: for each of 16 token tiles, lhsT(=x^T block, stationary)
@ rhs(=W^T slice, moving, N=512) bf16 matmuls accumulate fp32 into 4
PSUM banks (one per 512-wide output chunk) over the 16 k-blocks, so
each stationary load is amortized over 4 matmuls. The first two token
tiles are interleaved k-block-by-k-block so the PE chases the W-row DMA
arrivals during the prologue instead of idling. DVE/ACT split the PSUM
eviction per chunk; each chunk is DMA'd out as soon as it is evicted
(out-DMA triggers live on the gpsimd queue so they never block the
x-load queue).
"""

import os
import sys

import numpy as np

for _p in ("/opt/trn_rl_repo", "/root/.axon_site/_ro/trn_rl_repo"):
    if os.path.isdir(_p) and _p not in sys.path:
        sys.path.append(_p)

TOKENS, IN_F, OUT_F = 16384, 2048, 2048
NCORES = 8
TSH = TOKENS // NCORES  # tokens per core
P = 128
KB = IN_F // P  # contraction blocks
TB = TSH // P  # token tiles per core
OBL = OUT_F // P  # out_features blocks (scale granularity)
NCH = OUT_F // 512  # psum chunks of the output row-tile

_cached = None


def _build():
    from contextlib import ExitStack

    import concourse.tile as tile
    from concourse import bacc, mybir
    from concourse.bass import ds

    f32 = mybir.dt.float32
    bf16 = mybir.dt.bfloat16

    nc = bacc.Bacc("TRN2", target_bir_lowering=False, debug=False, num_devices=NCORES)
    xb_d = nc.dram_tensor("xb", [TB * P, IN_F], bf16, kind="ExternalInput").ap()
    wt_d = nc.dram_tensor("wt", [IN_F, OUT_F], bf16, kind="ExternalInput").ap()
    s_d = nc.dram_tensor("s", [P, KB, OBL], f32, kind="ExternalInput").ap()
    o_d = nc.dram_tensor("out", [TSH, OUT_F], f32, kind="ExternalOutput").ap()

    with tile.TileContext(nc) as tc:
        with ExitStack() as ctx:
            const = ctx.enter_context(tc.tile_pool(name="const", bufs=1))
            scales = const.tile([P, KB, OBL], f32)
            nc.scalar.dma_start(scales[:], s_d[:])

            wT_pool = ctx.enter_context(tc.tile_pool(name="wT", bufs=1))
            wTs = [wT_pool.tile([P, OUT_F], bf16, name=f"wT_{ib}") for ib in range(KB)]

            wnat_pool = ctx.enter_context(tc.tile_pool(name="wnat", bufs=1))
            x_pool = ctx.enter_context(tc.tile_pool(name="x", bufs=3))
            outsb_pool = ctx.enter_context(tc.tile_pool(name="outsb", bufs=2))
            ps_pool = ctx.enter_context(tc.tile_pool(name="ps", bufs=8, space="PSUM"))

            # Dequant engine split (full W staged resident, no buffer
            # recycling, so DMA never waits on dequant):
            #  - DVE: 10 rows at ~1.1ns/elem (needs a 2D contiguous in0 —
            #    a 3D-sliced in0 measured 2x slower)
            #  - GPSIMD: 4 rows (overhead-bound ~5us/row); these rows go
            #    LAST in every accumulation group so their lateness hides
            #  - ACT: 2 rows as 16 per-128-block muls with a per-partition
            #    [P,1] scale AP (ACT's scale can't vary along free dims)
            GPS_ROWS = (2, 6, 10, 14)
            ACT_ROWS = (5, 9)
            KB_ORDER = [kb for kb in range(KB) if kb not in GPS_ROWS] + list(GPS_ROWS)

            def dequant_row(kb, src):
                # src: 2D contiguous [P, OUT_F] bf16 view of W row kb
                if kb in ACT_ROWS:
                    for bo in range(OBL):
                        nc.scalar.mul(
                            wTs[kb][:, ds(bo * P, P)],
                            src[:, ds(bo * P, P)],
                            scales[:, kb, bo, None],
                        )
                    return
                eng = nc.gpsimd if kb in GPS_ROWS else nc.vector
                eng.tensor_tensor(
                    out=wTs[kb][:].rearrange("p (b c) -> p b c", c=P),
                    in0=src[:].rearrange("p (b c) -> p b c", c=P),
                    in1=scales[:, kb, ds(0, OBL), None].broadcast_to([P, OBL, P]),
                    op=mybir.AluOpType.mult,
                )

            def emit_w_row(kb, chunks=1):
                # triggers cost ~0.7us of issuing-engine time each, so even
                # rows issue from sync and odd rows from scalar in parallel
                trig = nc.sync if kb % 2 == 0 else nc.scalar
                wnat = wnat_pool.tile([P, OUT_F], bf16, name=f"wnat_{kb}")
                cw = OUT_F // chunks
                for j in range(chunks):
                    trig.dma_start(
                        wnat[:, ds(j * cw, cw)], wt_d[ds(kb * P, P), ds(j * cw, cw)]
                    )
                    if chunks > 1:
                        nb = cw // P
                        nc.vector.tensor_tensor(
                            out=wTs[kb][:, ds(j * cw, cw)].rearrange(
                                "p (b c) -> p b c", c=P
                            ),
                            in0=wnat[:, ds(j * cw, cw)].rearrange(
                                "p (b c) -> p b c", c=P
                            ),
                            in1=scales[:, kb, ds(j * nb, nb), None].broadcast_to(
                                [P, nb, P]
                            ),
                            op=mybir.AluOpType.mult,
                        )
                if chunks == 1:
                    dequant_row(kb, wnat[:])

            xtiles = {}

            def emit_x_tile(t):
                xt = x_pool.tile([P, IN_F], bf16, tag="x", name=f"x_{t}")
                nc.sync.dma_start(xt[:], xb_d[ds(t * P, P), :])
                xtiles[t] = xt

            psums = {}

            def open_group(t):
                psums[t] = [
                    ps_pool.tile([P, 512], f32, tag="ps", name=f"ps_{t}_{c}")
                    for c in range(NCH)
                ]

            def mm_one(t, i, c):
                kb = KB_ORDER[i]
                nc.tensor.matmul(
                    psums[t][c][:],
                    lhsT=xtiles[t][:, ds(kb * P, P)],
                    rhs=wTs[kb][:, ds(c * 512, 512)],
                    start=(i == 0),
                    stop=(i == KB - 1),
                )

            def mm(t, i):
                for c in range(NCH):
                    mm_one(t, i, c)

            def evict_chunk(t, c, outsb):
                if c % 2 == 0:
                    nc.vector.tensor_copy(outsb[:, ds(c * 512, 512)], psums[t][c][:])
                else:
                    nc.scalar.copy(outsb[:, ds(c * 512, 512)], psums[t][c][:])

            def close_tile(t):
                outsb = outsb_pool.tile([P, OUT_F], f32, tag="osb", name=f"osb_{t}")
                for c in range(NCH):
                    evict_chunk(t, c, outsb)
                # one trigger per tile (triggers cost ~0.7us of engine time
                # each) with 8KB DRAM runs
                nc.gpsimd.dma_start(o_d[ds(t * P, P), :], outsb[:])
                del psums[t]

            # ---- prologue: x0 first (gates the first matmul), then W rows
            # (row 0 quartered for an early start) on alternating trigger
            # engines; x3+ are gated by the bufs=3 pool so their packets
            # don't compete during the W-arrival window.
            emit_x_tile(0)
            emit_w_row(0, chunks=4)
            emit_x_tile(1)
            emit_w_row(1, chunks=2)
            for kb in range(2, KB):
                emit_w_row(kb)
            emit_x_tile(2)
            emit_x_tile(3)

            # ---- first two token tiles interleaved k-block-by-k-block so the
            # PE has 2 tiles' worth of matmuls (~1.7us) per W-row arrival
            # (~1.5us) and never starves during the W load phase.
            open_group(0)
            open_group(1)
            for i in range(KB):
                mm(0, i)
                mm(1, i)
            close_tile(0)
            close_tile(1)

            # ---- steady state: pure matmul stream, x prefetched 2 ahead ----
            for t in range(2, TB - 1):
                open_group(t)
                if t + 2 < TB:
                    emit_x_tile(t + 2)
                for i in range(KB):
                    mm(t, i)
                close_tile(t)

            # ---- last tile: chunk-outer so each chunk's eviction + out-DMA
            # pipelines under the next chunk's matmuls, shrinking the tail
            t = TB - 1
            open_group(t)
            outsb = outsb_pool.tile([P, OUT_F], f32, tag="osb", name=f"osb_{t}")
            for c in range(NCH):
                for i in range(KB):
                    mm_one(t, i, c)
                evict_chunk(t, c, outsb)
                eng = nc.gpsimd if c % 2 == 0 else nc.scalar
                eng.dma_start(
                    o_d[ds(t * P, P), ds(c * 512, 512)], outsb[:, ds(c * 512, 512)]
                )
            del psums[t]

    nc.compile()
    return nc


def _get_compiled():
    global _cached
    if _cached is None:
        _cached = _build()
    return _cached


def _ensure_ntff_hook():
    """Register the axon NTFF profile hook (boot skips it when
    antenv.axon_hooks is absent from the image). Only needed for trace=True."""
    import sys as _sys
    import types as _types

    if "antenv.axon_hooks" not in _sys.modules:
        import antenv

        mod = _types.ModuleType("antenv.axon_hooks")
        mod._hook = None

        def set_axon_ntff_profile_hook(h):
            mod._hook = h

        def get_axon_ntff_profile_hook():
            return mod._hook

        mod.set_axon_ntff_profile_hook = set_axon_ntff_profile_hook
        mod.get_axon_ntff_profile_hook = get_axon_ntff_profile_hook
        _sys.modules["antenv.axon_hooks"] = mod
        antenv.axon_hooks = mod
    mod = _sys.modules["antenv.axon_hooks"]
    if mod._hook is None:
        from trn_agent_boot.trn_boot import _ntff_profile_via_ctypes

        hook = _ntff_profile_via_ctypes("/opt/axon/libaxon_pjrt.so")
        if hook is not None:
            mod.set_axon_ntff_profile_hook(hook)


def run(x, weight, weight_scale, trace=False, trace_cores=None):
    from concourse.bass_utils import run_bass_kernel_spmd

    nc = _get_compiled()

    import ml_dtypes

    bf16 = ml_dtypes.bfloat16
    x = np.asarray(x, dtype=np.float32)
    weight = np.asarray(weight, dtype=np.float32)
    wt = np.ascontiguousarray(weight.T.astype(bf16))
    weight_scale = np.asarray(weight_scale, dtype=np.float32)
    # [P, KB(bi), OBL(bo)]: s[p, bi, bo] = weight_scale[bo, bi]
    scales_b = np.ascontiguousarray(
        np.broadcast_to(weight_scale.T[None, :, :], (P, KB, OBL)).astype(np.float32)
    )

    def blocked_x(shard):
        # xb[t, p, kb, m] = shard[t*128+m, kb*128+p]  (layout prep only)
        xb = shard.reshape(TB, P, KB, P).transpose(0, 3, 2, 1)
        return np.ascontiguousarray(xb.astype(bf16).reshape(TB * P, IN_F))

    in_maps = [
        {
            "xb": blocked_x(x[c * TSH : (c + 1) * TSH]),
            "wt": wt,
            "s": scales_b,
        }
        for c in range(NCORES)
    ]
    kwargs = {}
    if trace:
        try:
            _ensure_ntff_hook()
        except Exception as e:  # tracing is best-effort; the run still works
            print(f"ntff hook registration failed ({e}); tracing may be skipped")
        kwargs = dict(trace=True, trace_cores=trace_cores or [0])
    res = run_bass_kernel_spmd(nc, in_maps, core_ids=list(range(NCORES)), **kwargs)
    out = np.concatenate([res.results[c]["out"] for c in range(NCORES)], axis=0)
    return out, res


def kernel(x, weight, weight_scale):
    # Rare transient device errors (NRT_EXEC_UNIT_UNRECOVERABLE) have been
    # observed under the profiling path; retry once to be safe.
    try:
        out, _ = run(x, weight, weight_scale)
    except Exception:
        import time

        time.sleep(2)
        out, _ = run(x, weight, weight_scale)
    return out
